# revision 1
# baseline (speedup 1.0000x reference)
"""GAT-style 2-layer GNN message passing on 8 Trainium2 NeuronCores.

Math note: for this reference, the segment-softmax ratio
  num/den = (sum_j h[j]*exp((s_l[i]+s_r[j])/2d)) / (sum_j exp((s_l[i]+s_r[j])/2d))
has the destination factor exp(s_l[i]/2d) cancel, so per layer we only need
  a[i] = (sum_{j in N(i)} w_j*h_j) / (sum_{j in N(i)} w_j),  w_j = exp(s_r[j]/2d).

Sharding: nodes split into 8 contiguous destination ranges (6250/core).
Each core builds table rows [g=w*h (256) | w (4) | pad] (bf16, 768B) for its
own nodes, an AllGather shares the full 50k-row table, then each core
aggregates its own destinations: per-edge dma_gather of source rows + one-hot
matmul (segment-sum into PSUM), then divide / layernorm / leaky-relu.
Edges are pre-sorted by destination on the host; indices are int16 so the
table is gathered via two base pointers (src < 32768 and src >= 32768).
"""

import os
import sys

import numpy as np
import ml_dtypes

sys.path.insert(0, "/opt/trn_rl_repo")

import concourse.bacc as bacc
import concourse.bass as bass
import concourse.mybir as mybir
import concourse.tile as tile
from concourse.bass_utils import run_bass_kernel_spmd

BF16 = mybir.dt.bfloat16
F32 = mybir.dt.float32
I16 = mybir.dt.int16

N, DIN, E = 50000, 128, 800000
H, D = 4, 64
F = H * D  # 256
NCORE = 8
NPC = N // NCORE  # 6250
NBLK = (NPC + 127) // 128  # 49 destination blocks per core
EPS = 1e-5
SLOPE = 0.01
ROWE = 384  # table row: 256 g + 4 w + 124 pad (bf16) = 768 bytes
SPLIT = 32768  # int16 index limit

LAST_RESULTS = None

AF = mybir.ActivationFunctionType
ALU = mybir.AluOpType


def _host_prep(x, edge_index, W0, b0, W1, b1, att0, att1, gamma, beta):
    """Build all per-core and shared input arrays."""
    bf16 = ml_dtypes.bfloat16
    dst = np.asarray(edge_index[0], dtype=np.int64)
    src = np.asarray(edge_index[1], dtype=np.int64)
    x = np.asarray(x, dtype=np.float32)

    # --- per-core edge grids ---
    per_core = []
    nb1_max, nb2_max = 0, 0
    for c in range(NCORE):
        lo_node, hi_node = c * NPC, (c + 1) * NPC
        m = (dst >= lo_node) & (dst < hi_node)
        ld = (dst[m] - lo_node).astype(np.int64)
        s = src[m]
        blocks = []
        for b in range(NBLK):
            bm = (ld >> 7) == b
            sb = s[bm]
            lb = (ld[bm] & 127).astype(np.int64)
            lo_m = sb < SPLIT
            runs = (
                (sb[lo_m], lb[lo_m]),
                (sb[~lo_m] - SPLIT, lb[~lo_m]),
            )
            blocks.append(runs)
            nb1_max = max(nb1_max, -(-len(runs[0][0]) // 128))
            nb2_max = max(nb2_max, -(-len(runs[1][0]) // 128))
        per_core.append(blocks)
    NB1, NB2 = max(nb1_max, 1), max(nb2_max, 1)
    NBT = NB1 + NB2

    eidx_all, dloc_all = [], []
    for c in range(NCORE):
        eidx = np.zeros((128, NBLK * NBT * 8), dtype=np.int16)
        dloc = np.full((128, NBLK * NBT), 255.0, dtype=bf16)
        for b in range(NBLK):
            for r, nb in ((0, NB1), (1, NB2)):
                sidx, lb = per_core[c][b][r]
                n = len(sidx)
                pad = nb * 128 - n
                si = np.concatenate([sidx, np.zeros(pad, np.int64)]).astype(np.int16)
                dl = np.concatenate([lb, np.full(pad, 255, np.int64)]).astype(
                    np.float32
                )
                c0 = (b * NBT + (0 if r == 0 else NB1)) * 8
                eidx[:, c0 : c0 + nb * 8] = np.tile(
                    si.reshape(nb * 8, 16).T, (8, 1)
                )
                cb = b * NBT + (0 if r == 0 else NB1)
                dloc[:, cb : cb + nb] = dl.reshape(nb, 128).T.astype(bf16)
        eidx_all.append(eidx)
        dloc_all.append(dloc)

    # --- xT slices (bf16, padded to 49*128 columns) ---
    xts = []
    for c in range(NCORE):
        xt = np.zeros((DIN, NBLK * 128), dtype=bf16)
        xt[:, :NPC] = x[c * NPC : (c + 1) * NPC].T.astype(bf16)
        xts.append(xt)

    shared = {
        "w0t": np.ascontiguousarray(np.asarray(W0, np.float32).T).astype(bf16),
        "w1t": np.ascontiguousarray(np.asarray(W1, np.float32).T).astype(bf16),
        "b0b": np.tile(np.asarray(b0, np.float32)[None, :], (128, 1)),
        "b1b": np.tile(np.asarray(b1, np.float32)[None, :], (128, 1)),
        "ar0": np.tile(
            np.asarray(att0, np.float32)[0, :, D:].reshape(-1)[None, :], (128, 1)
        ),
        "ar1": np.tile(
            np.asarray(att1, np.float32)[0, :, D:].reshape(-1)[None, :], (128, 1)
        ),
        "gmb": np.tile(np.asarray(gamma, np.float32)[None, :], (128, 1)),
        "btb": np.tile(np.asarray(beta, np.float32)[None, :], (128, 1)),
        "iot": np.tile(
            np.tile(np.arange(128, dtype=np.float32), max(NB1, NB2))[None, :],
            (128, 1),
        ).astype(bf16),
        "idn": np.eye(128, dtype=np.float32).astype(bf16),
    }
    in_maps = []
    for c in range(NCORE):
        m = dict(shared)
        m["xt"] = xts[c]
        m["eidx"] = eidx_all[c]
        m["dloc"] = dloc_all[c]
        in_maps.append(m)
    return in_maps, NB1, NB2


def _build_program(NB1, NB2):
    NBT = NB1 + NB2
    nc = bacc.Bacc("TRN2", target_bir_lowering=False, debug=False, num_devices=NCORE)

    # I/O
    xt_d = nc.dram_tensor("xt", [DIN, NBLK * 128], BF16, kind="ExternalInput")
    eidx_d = nc.dram_tensor("eidx", [128, NBLK * NBT * 8], I16, kind="ExternalInput")
    dloc_d = nc.dram_tensor("dloc", [128, NBLK * NBT], BF16, kind="ExternalInput")
    w0t_d = nc.dram_tensor("w0t", [DIN, F], BF16, kind="ExternalInput")
    w1t_d = nc.dram_tensor("w1t", [F, F], BF16, kind="ExternalInput")
    b0b_d = nc.dram_tensor("b0b", [128, F], F32, kind="ExternalInput")
    b1b_d = nc.dram_tensor("b1b", [128, F], F32, kind="ExternalInput")
    ar0_d = nc.dram_tensor("ar0", [128, F], F32, kind="ExternalInput")
    ar1_d = nc.dram_tensor("ar1", [128, F], F32, kind="ExternalInput")
    gmb_d = nc.dram_tensor("gmb", [128, F], F32, kind="ExternalInput")
    btb_d = nc.dram_tensor("btb", [128, F], F32, kind="ExternalInput")
    iot_d = nc.dram_tensor("iot", [128, max(NB1, NB2) * 128], BF16, kind="ExternalInput")
    idn_d = nc.dram_tensor("idn", [128, 128], BF16, kind="ExternalInput")
    out_d = nc.dram_tensor("out", [NPC, D], F32, kind="ExternalOutput")

    # Internal DRAM
    tbl_own = [nc.dram_tensor(f"tbl_own{l}", [NPC, ROWE], BF16) for l in range(2)]
    tbl_full = [
        nc.dram_tensor(f"tbl_full{l}", [N, ROWE], BF16, addr_space="Shared")
        for l in range(2)
    ]

    groups = [list(range(NCORE))]

    with tile.TileContext(nc) as tc:
        with (
            tc.tile_pool(name="const", bufs=1) as cpool,
            tc.tile_pool(name="stat", bufs=3) as spool,
            tc.tile_pool(name="hbuf", bufs=3) as hpool,
            tc.tile_pool(name="small", bufs=4) as smpool,
            tc.tile_pool(name="tblt", bufs=3) as tbpool,
            tc.tile_pool(name="glo", bufs=2) as glopool,
            tc.tile_pool(name="ghi", bufs=2) as ghipool,
            tc.tile_pool(name="oh", bufs=4) as ohpool,
            tc.tile_pool(name="post", bufs=3) as postpool,
            tc.tile_pool(name="gemm", bufs=2, space="PSUM") as gpsum,
            tc.tile_pool(name="agg", bufs=2, space="PSUM") as apsum,
            tc.tile_pool(name="tp", bufs=2, space="PSUM") as tpsum,
        ):
            # ---- load constants ----
            def cload(dram, shape, dtype):
                t = cpool.tile(shape, dtype, tag=dram.name)
                nc.sync.dma_start(out=t[:], in_=dram[:, :])
                return t

            xt_s = cload(xt_d, [DIN, NBLK * 128], BF16)
            eidx_s = cload(eidx_d, [128, NBLK * NBT * 8], I16)
            dloc_s = cload(dloc_d, [128, NBLK * NBT], BF16)
            w0t_s = cload(w0t_d, [DIN, F], BF16)
            w1t_s = []
            for cch in range(2):
                t = cpool.tile([128, F], BF16, tag=f"w1t{cch}")
                nc.sync.dma_start(
                    out=t[:], in_=w1t_d[cch * 128 : (cch + 1) * 128, :]
                )
                w1t_s.append(t)
            b0b_s = cload(b0b_d, [128, F], F32)
            b1b_s = cload(b1b_d, [128, F], F32)
            ar0_s = cload(ar0_d, [128, F], F32)
            ar1_s = cload(ar1_d, [128, F], F32)
            gmb_s = cload(gmb_d, [128, F], F32)
            btb_s = cload(btb_d, [128, F], F32)
            iot_s = cload(iot_d, [128, max(NB1, NB2) * 128], BF16)
            idn_s = cload(idn_d, [128, 128], BF16)
            x1_s = cpool.tile([128, NBLK * F], BF16, tag="x1")
            gt_bufs = {}
            for r, nb in ((0, NB1), (1, NB2)):
                for i in range(2):
                    t = cpool.tile([128, nb, ROWE], BF16, tag=f"gt{r}_{i}")
                    nc.vector.memset(t[:], 0.0)
                    gt_bufs[(r, i)] = t

            def build_layer(l):
                b_s = b0b_s if l == 0 else b1b_s
                ar_s = ar0_s if l == 0 else ar1_s
                for t in range(NBLK):
                    rows = min(128, NPC - t * 128)
                    ps = gpsum.tile([128, F], F32, tag="gemm")
                    if l == 0:
                        nc.tensor.matmul(
                            ps[:],
                            lhsT=xt_s[:, t * 128 : (t + 1) * 128],
                            rhs=w0t_s[:],
                            start=True,
                            stop=True,
                        )
                    else:
                        for cch in range(2):
                            pt = tpsum.tile([128, 128], BF16, tag="tp")
                            nc.tensor.transpose(
                                pt[:],
                                x1_s[:, t * F + cch * 128 : t * F + (cch + 1) * 128],
                                idn_s[:],
                            )
                            st = spool.tile([128, 128], BF16, tag="stat")
                            nc.vector.tensor_copy(st[:], pt[:])
                            nc.tensor.matmul(
                                ps[:],
                                lhsT=st[:],
                                rhs=w1t_s[cch][:],
                                start=(cch == 0),
                                stop=(cch == 1),
                            )
                    h = hpool.tile([128, F], F32, tag="h")
                    nc.vector.tensor_tensor(h[:], ps[:], b_s[:], op=ALU.add)
                    u = hpool.tile([128, F], F32, tag="u")
                    nc.vector.tensor_tensor(u[:], h[:], ar_s[:], op=ALU.mult)
                    # lrelu(v) = (1+SLOPE)/2*v + (1-SLOPE)/2*|v|; sum over d per head
                    su = smpool.tile([128, H], F32, tag="su")
                    nc.vector.tensor_reduce(
                        su[:],
                        u[:].rearrange("p (h d) -> p h d", d=D),
                        axis=mybir.AxisListType.X,
                        op=ALU.add,
                    )
                    sa = smpool.tile([128, H], F32, tag="sa")
                    nc.vector.tensor_reduce(
                        sa[:],
                        u[:].rearrange("p (h d) -> p h d", d=D),
                        axis=mybir.AxisListType.X,
                        op=ALU.add,
                        apply_absolute_value=True,
                    )
                    sr = smpool.tile([128, H], F32, tag="sr")
                    nc.vector.tensor_scalar(
                        sr[:], su[:], (1 + SLOPE) / 2 / (2 * D), None, op0=ALU.mult
                    )
                    sr2 = smpool.tile([128, H], F32, tag="sr2")
                    nc.vector.tensor_scalar(
                        sr2[:], sa[:], (1 - SLOPE) / 2 / (2 * D), None, op0=ALU.mult
                    )
                    srf = smpool.tile([128, H], F32, tag="srf")
                    nc.vector.tensor_tensor(srf[:], sr[:], sr2[:], op=ALU.add)
                    wv = smpool.tile([128, H], F32, tag="wv")
                    nc.scalar.activation(wv[:], srf[:], AF.Exp)
                    tb = tbpool.tile([128, F + H], BF16, tag="tb")
                    for hd in range(H):
                        nc.vector.tensor_scalar_mul(
                            tb[:, hd * D : (hd + 1) * D],
                            h[:, hd * D : (hd + 1) * D],
                            wv[:, hd : hd + 1],
                        )
                    nc.vector.tensor_copy(tb[:, F : F + H], wv[:])
                    nc.sync.dma_start(
                        out=tbl_own[l][t * 128 : t * 128 + rows, 0 : F + H],
                        in_=tb[:rows, :],
                    )
                nc.gpsimd.collective_compute(
                    "AllGather",
                    ALU.bypass,
                    replica_groups=groups,
                    ins=[tbl_own[l][:, :]],
                    outs=[tbl_full[l][:, :]],
                )

            def agg_layer(l):
                for t in range(NBLK):
                    rows = min(128, NPC - t * 128)
                    ps = apsum.tile([128, F + H], F32, tag="agg")
                    nmm = 0
                    for r, nb in ((0, NB1), (1, NB2)):
                        gt = gt_bufs[(r, t % 2)]
                        base = (
                            tbl_full[l][0:SPLIT, :]
                            if r == 0
                            else tbl_full[l][SPLIT:N, :]
                        )
                        c0 = (t * NBT + (0 if r == 0 else NB1)) * 8
                        nc.gpsimd.dma_gather(
                            gt[:],
                            base,
                            eidx_s[:, c0 : c0 + nb * 8],
                            nb * 128,
                            nb * 128,
                            ROWE,
                            single_packet=(nb * 128 <= 1024),
                        )
                        cb = t * NBT + (0 if r == 0 else NB1)
                        oh = ohpool.tile([128, nb * 128], BF16, tag=f"oh{r}")
                        nc.vector.tensor_tensor(
                            oh[:].rearrange("p (a b) -> p a b", b=128),
                            iot_s[:, 0 : nb * 128].rearrange(
                                "p (a b) -> p a b", b=128
                            ),
                            dloc_s[:, cb : cb + nb].to_broadcast((128, nb, 128)),
                            op=ALU.is_equal,
                        )
                        for b in range(nb):
                            nc.tensor.matmul(
                                ps[:],
                                lhsT=oh[:, b * 128 : (b + 1) * 128],
                                rhs=gt[:, b, 0 : F + H],
                                start=(nmm == 0),
                                stop=(nmm == NBT - 1),
                            )
                            nmm += 1
                    rec = smpool.tile([128, H], F32, tag="rec")
                    nc.vector.reciprocal(rec[:], ps[:, F : F + H])
                    if l == 0:
                        a0 = postpool.tile([128, F], F32, tag="a0")
                        for hd in range(H):
                            nc.vector.tensor_scalar_mul(
                                a0[:, hd * D : (hd + 1) * D],
                                ps[:, hd * D : (hd + 1) * D],
                                rec[:, hd : hd + 1],
                            )
                        mu = smpool.tile([128, 1], F32, tag="mu")
                        nc.vector.tensor_reduce(
                            mu[:], a0[:], axis=mybir.AxisListType.X, op=ALU.add
                        )
                        nc.vector.tensor_scalar_mul(mu[:], mu[:], 1.0 / F)
                        dd = postpool.tile([128, F], F32, tag="dd")
                        nc.vector.tensor_scalar_sub(dd[:], a0[:], mu[:])
                        vs = smpool.tile([128, 1], F32, tag="vs")
                        scr2 = postpool.tile([128, F], F32, tag="scr2")
                        nc.scalar.activation(
                            scr2[:], dd[:], AF.Square, accum_out=vs[:]
                        )
                        vs2 = smpool.tile([128, 1], F32, tag="vs2")
                        nc.vector.tensor_scalar(
                            vs2[:], vs[:], 1.0 / F, EPS, op0=ALU.mult, op1=ALU.add
                        )
                        sd = smpool.tile([128, 1], F32, tag="sd")
                        nc.scalar.activation(sd[:], vs2[:], AF.Sqrt)
                        rstd = smpool.tile([128, 1], F32, tag="rstd")
                        nc.vector.reciprocal(rstd[:], sd[:])
                        xn = postpool.tile([128, F], F32, tag="xn")
                        nc.vector.tensor_scalar_mul(xn[:], dd[:], rstd[:])
                        xg = postpool.tile([128, F], F32, tag="xg")
                        nc.vector.tensor_tensor(xg[:], xn[:], gmb_s[:], op=ALU.mult)
                        xgb = postpool.tile([128, F], F32, tag="xgb")
                        nc.vector.tensor_tensor(xgb[:], xg[:], btb_s[:], op=ALU.add)
                        nc.scalar.activation(
                            x1_s[:, t * F : (t + 1) * F],
                            xgb[:],
                            AF.Lrelu,
                            alpha=SLOPE,
                        )
                    else:
                        q = postpool.tile([128, F], F32, tag="a0")
                        for hd in range(H):
                            nc.vector.tensor_scalar_mul(
                                q[:, hd * D : (hd + 1) * D],
                                ps[:, hd * D : (hd + 1) * D],
                                rec[:, hd : hd + 1],
                            )
                        p01 = postpool.tile([128, D], F32, tag="p01")
                        p23 = postpool.tile([128, D], F32, tag="p23")
                        nc.vector.tensor_tensor(
                            p01[:], q[:, 0:D], q[:, D : 2 * D], op=ALU.add
                        )
                        nc.vector.tensor_tensor(
                            p23[:], q[:, 2 * D : 3 * D], q[:, 3 * D : 4 * D], op=ALU.add
                        )
                        o = postpool.tile([128, D], F32, tag="o")
                        nc.vector.tensor_tensor(o[:], p01[:], p23[:], op=ALU.add)
                        nc.vector.tensor_scalar_mul(o[:], o[:], 0.25)
                        nc.sync.dma_start(
                            out=out_d[t * 128 : t * 128 + rows, :], in_=o[:rows, :]
                        )

            phases = os.environ.get("KPHASES", "b0,a0,b1,a1").split(",")
            if "b0" in phases:
                build_layer(0)
            if "a0" in phases:
                agg_layer(0)
            if "b1" in phases:
                build_layer(1)
            if "a1" in phases:
                agg_layer(1)
            if "a1" not in phases:
                # dummy output so the ExternalOutput is written
                zt = postpool.tile([128, D], F32, tag="o")
                nc.vector.memset(zt[:], 0.0)
                nc.sync.dma_start(out=out_d[0:128, :], in_=zt[:])

    nc.compile()
    return nc


_CACHE = {}


def kernel(**inputs):
    global LAST_RESULTS
    in_maps, NB1, NB2 = _host_prep(**inputs)
    key = (NB1, NB2, os.environ.get("KPHASES", "b0,a0,b1,a1"))
    if key not in _CACHE:
        _CACHE[key] = _build_program(NB1, NB2)
    nc = _CACHE[key]
    trace = bool(os.environ.get("BASS_TRACE"))
    res = run_bass_kernel_spmd(nc, in_maps, list(range(NCORE)), trace=trace)
    LAST_RESULTS = res
    out = np.concatenate([res.results[c]["out"] for c in range(NCORE)], axis=0)
    return out.astype(np.float32)



# revision 9
# speedup vs baseline: 1.5318x; 1.5318x over previous
"""GAT-style 2-layer GNN message passing on 8 Trainium2 NeuronCores.

Math note: for this reference, the segment-softmax ratio
  num/den = (sum_j h[j]*exp((s_l[i]+s_r[j])/2d)) / (sum_j exp((s_l[i]+s_r[j])/2d))
has the destination factor exp(s_l[i]/2d) cancel, so per layer we only need
  a[i] = (sum_{j in N(i)} w_j*h_j) / (sum_{j in N(i)} w_j),  w_j = exp(s_r[j]/2d).

Sharding: nodes split into 8 contiguous destination ranges (6250/core).
Each core builds table rows [g=w*h (256) | w (4) | pad] (bf16, 768B) for its
own nodes. The table is split into two halves by local row (A: rows 0..3199,
B: rows 3200..6249) and distributed with two pipelined AllGathers per layer.
Each core then aggregates its own destinations: per-edge dma_gather of source
rows (spread over the 4 SWDGE queues so descriptor generation runs on all 8
Q7 cores) + one-hot matmul segment-sum into PSUM (one-hot matrices are
precomputed on the host and streamed from DRAM), then divide / layernorm /
leaky-relu. Edge index lists are padded with -1 so the gather ucode trims
trailing entries and only real edges cost descriptors.
"""

import os
import sys

import numpy as np
import ml_dtypes

sys.path.insert(0, "/opt/trn_rl_repo")

import concourse.bacc as bacc
import concourse.bass as bass
import concourse.mybir as mybir
import concourse.tile as tile
from concourse.bass_utils import run_bass_kernel_spmd

BF16 = mybir.dt.bfloat16
F32 = mybir.dt.float32
I16 = mybir.dt.int16

N, DIN, E = 50000, 128, 800000
H, D = 4, 64
F = H * D  # 256
NCORE = 8
NPC = N // NCORE  # 6250
NBLK = (NPC + 127) // 128  # 49 destination blocks per core
EPS = 1e-5
SLOPE = 0.01
ROWE = 384  # table row: 256 g + 4 w + 124 pad (bf16) = 768 bytes
SPLITA = 3200  # local rows 0..3199 -> table A (25 blocks), rest -> table B
SPLITB = NPC - SPLITA  # 3050
K1 = (1 + SLOPE) / 2 / (2 * D)
K2 = (1 - SLOPE) / 2 / (2 * D)

LAST_RESULTS = None

AF = mybir.ActivationFunctionType
ALU = mybir.AluOpType


def _host_prep(x, edge_index, W0, b0, W1, b1, att0, att1, gamma, beta):
    """Build all per-core and shared input arrays."""
    bf16 = ml_dtypes.bfloat16
    dst = np.asarray(edge_index[0], dtype=np.int64)
    src = np.asarray(edge_index[1], dtype=np.int64)
    x = np.asarray(x, dtype=np.float32)

    plain_ln = bool(
        np.allclose(np.asarray(gamma), 1.0) and np.allclose(np.asarray(beta), 0.0)
    )

    # --- per-core edge lists, split by source-owner local row half ---
    per_core = []  # [c][b] -> ((idxA, lbA), (idxB, lbB))
    nba_max, nbb_max = 1, 1
    for c in range(NCORE):
        m = (dst >= c * NPC) & (dst < (c + 1) * NPC)
        ld = dst[m] - c * NPC
        s = src[m]
        owner = s // NPC
        srow = s - owner * NPC
        inA = srow < SPLITA
        idxA_all = owner * SPLITA + srow
        idxB_all = owner * SPLITB + (srow - SPLITA)
        blocks = []
        for b in range(NBLK):
            bm = (ld >> 7) == b
            lb = ld[bm] & 127
            a_m = inA[bm]
            runs = (
                (idxA_all[bm][a_m], lb[a_m]),
                (idxB_all[bm][~a_m], lb[~a_m]),
            )
            blocks.append(runs)
            nba_max = max(nba_max, -(-len(runs[0][0]) // 128))
            nbb_max = max(nbb_max, -(-len(runs[1][0]) // 128))
        per_core.append(blocks)
    NBA, NBB = nba_max, nbb_max
    NBT = NBA + NBB

    eidxA_all, eidxB_all, ohm_all = [], [], []
    for c in range(NCORE):
        eidxA = np.full((128, NBLK * NBA * 8), -1, dtype=np.int16)
        eidxB = np.full((128, NBLK * NBB * 8), -1, dtype=np.int16)
        ohm = np.zeros((128, NBLK * NBT * 128), dtype=bf16)
        for b in range(NBLK):
            for r, nb, eidx in ((0, NBA, eidxA), (1, NBB, eidxB)):
                sidx, lb = per_core[c][b][r]
                n = len(sidx)
                pad = nb * 128 - n
                padval = -1 if int(os.environ.get("KTRIM", "0")) else 0
                si = np.concatenate(
                    [sidx, np.full(pad, padval, np.int64)]
                ).astype(np.int16)
                eidx[:, b * nb * 8 : (b + 1) * nb * 8] = np.tile(
                    si.reshape(nb * 8, 16).T, (8, 1)
                )
                # one-hot lhsT layout: partition = edge position within its
                # 128-chunk, free column = chunk*128 + destination slot.
                col0 = b * NBT * 128 + (0 if r == 0 else NBA * 128)
                pos = np.arange(n)
                ohm[pos % 128, col0 + (pos // 128) * 128 + lb.astype(np.int64)] = 1.0
        eidxA_all.append(eidxA)
        eidxB_all.append(eidxB)
        ohm_all.append(ohm)

    # --- xT slices (bf16, padded to 49*128 columns) ---
    xts = []
    for c in range(NCORE):
        xt = np.zeros((DIN, NBLK * 128), dtype=bf16)
        xt[:, :NPC] = x[c * NPC : (c + 1) * NPC].T.astype(bf16)
        xts.append(xt)

    shared = {
        "w0t": np.ascontiguousarray(np.asarray(W0, np.float32).T).astype(bf16),
        "w1t": np.ascontiguousarray(np.asarray(W1, np.float32).T).astype(bf16),
        "b0b": np.tile(np.asarray(b0, np.float32)[None, :], (128, 1)),
        "b1b": np.tile(np.asarray(b1, np.float32)[None, :], (128, 1)),
        "ar0": np.tile(
            np.asarray(att0, np.float32)[0, :, D:].reshape(-1)[None, :], (128, 1)
        ),
        "ar1": np.tile(
            np.asarray(att1, np.float32)[0, :, D:].reshape(-1)[None, :], (128, 1)
        ),
        "gmb": np.tile(np.asarray(gamma, np.float32)[None, :], (128, 1)),
        "btb": np.tile(np.asarray(beta, np.float32)[None, :], (128, 1)),
        "idn": np.eye(128, dtype=np.float32).astype(bf16),
    }
    in_maps = []
    for c in range(NCORE):
        m = dict(shared)
        m["xt"] = xts[c]
        m["eidxA"] = eidxA_all[c]
        m["eidxB"] = eidxB_all[c]
        m["ohm"] = ohm_all[c]
        in_maps.append(m)
    return in_maps, NBA, NBB, plain_ln


def _build_program(NBA, NBB, plain_ln):
    NBT = NBA + NBB
    KQ4 = int(os.environ.get("KQ4", "1"))  # spread gathers over 4 SWDGE queues
    KRAF = int(os.environ.get("KRAF", "1"))  # use reciprocal_approx_fast
    KSTR = int(os.environ.get("KSTR", "1"))  # strided head-mean reduce
    KPH = os.environ.get("KPH", "b0,a0,b1,a1").split(",")
    nc = bacc.Bacc(
        "TRN2",
        target_bir_lowering=False,
        debug=False,
        num_devices=NCORE,
        num_swdge_queues=4 if KQ4 else 1,
    )

    # I/O
    xt_d = nc.dram_tensor("xt", [DIN, NBLK * 128], BF16, kind="ExternalInput")
    eidxA_d = nc.dram_tensor("eidxA", [128, NBLK * NBA * 8], I16, kind="ExternalInput")
    eidxB_d = nc.dram_tensor("eidxB", [128, NBLK * NBB * 8], I16, kind="ExternalInput")
    ohm_d = nc.dram_tensor("ohm", [128, NBLK * NBT * 128], BF16, kind="ExternalInput")
    w0t_d = nc.dram_tensor("w0t", [DIN, F], BF16, kind="ExternalInput")
    w1t_d = nc.dram_tensor("w1t", [F, F], BF16, kind="ExternalInput")
    b0b_d = nc.dram_tensor("b0b", [128, F], F32, kind="ExternalInput")
    b1b_d = nc.dram_tensor("b1b", [128, F], F32, kind="ExternalInput")
    ar0_d = nc.dram_tensor("ar0", [128, F], F32, kind="ExternalInput")
    ar1_d = nc.dram_tensor("ar1", [128, F], F32, kind="ExternalInput")
    gmb_d = nc.dram_tensor("gmb", [128, F], F32, kind="ExternalInput")
    btb_d = nc.dram_tensor("btb", [128, F], F32, kind="ExternalInput")
    idn_d = nc.dram_tensor("idn", [128, 128], BF16, kind="ExternalInput")
    out_d = nc.dram_tensor("out", [NPC, D], F32, kind="ExternalOutput")

    # Internal DRAM
    tblA_own = [nc.dram_tensor(f"tblA_own{l}", [SPLITA, ROWE], BF16) for l in range(2)]
    tblB_own = [nc.dram_tensor(f"tblB_own{l}", [SPLITB, ROWE], BF16) for l in range(2)]
    tblA_full = [
        nc.dram_tensor(f"tblA_full{l}", [NCORE * SPLITA, ROWE], BF16, addr_space="Shared")
        for l in range(2)
    ]
    tblB_full = [
        nc.dram_tensor(f"tblB_full{l}", [NCORE * SPLITB, ROWE], BF16, addr_space="Shared")
        for l in range(2)
    ]

    groups = [list(range(NCORE))]
    NBLKA = SPLITA // 128  # 25

    with tile.TileContext(nc) as tc:
        with (
            tc.tile_pool(name="const", bufs=1) as cpool,
            tc.tile_pool(name="stat", bufs=4) as spool,
            tc.tile_pool(name="hbuf", bufs=3) as hpool,
            tc.tile_pool(name="small", bufs=6) as smpool,
            tc.tile_pool(name="tblt", bufs=3) as tbpool,
            tc.tile_pool(name="ohp", bufs=3) as ohpool,
            tc.tile_pool(name="post", bufs=3) as postpool,
            tc.tile_pool(name="gemm", bufs=2, space="PSUM") as gpsum,
            tc.tile_pool(name="agg", bufs=3, space="PSUM") as apsum,
            tc.tile_pool(name="tp", bufs=2, space="PSUM") as tpsum,
        ):
            # ---- load constants ----
            def cload(dram, shape, dtype):
                t = cpool.tile(shape, dtype, tag=dram.name)
                nc.sync.dma_start(out=t[:], in_=dram[:, :])
                return t

            xt_s = cload(xt_d, [DIN, NBLK * 128], BF16)
            eidxA_s = cload(eidxA_d, [128, NBLK * NBA * 8], I16)
            eidxB_s = cload(eidxB_d, [128, NBLK * NBB * 8], I16)
            w0t_s = cload(w0t_d, [DIN, F], BF16)
            w1t_s = []
            for cch in range(2):
                t = cpool.tile([128, F], BF16, tag=f"w1t{cch}")
                nc.sync.dma_start(
                    out=t[:], in_=w1t_d[cch * 128 : (cch + 1) * 128, :]
                )
                w1t_s.append(t)
            b0b_s = cload(b0b_d, [128, F], F32)
            b1b_s = cload(b1b_d, [128, F], F32)
            ar0_s = cload(ar0_d, [128, F], F32)
            ar1_s = cload(ar1_d, [128, F], F32)
            if not plain_ln:
                gmb_s = cload(gmb_d, [128, F], F32)
                btb_s = cload(btb_d, [128, F], F32)
            idn_s = cload(idn_d, [128, 128], BF16)
            epsb_s = cpool.tile([128, 1], F32, tag="epsb")
            nc.vector.memset(epsb_s[:], EPS)
            x1_s = cpool.tile([128, NBLK * F], BF16, tag="x1")
            h1b_s = cpool.tile([128, NBLK * F], BF16, tag="h1b")
            srf1_s = cpool.tile([128, NBLK * H], F32, tag="srf1")
            gtA = []
            gtB = []
            for i in range(3):
                t = cpool.tile([128, NBA, ROWE], BF16, tag=f"gtA{i}")
                nc.vector.memset(t[:], 0.0)
                gtA.append(t)
                t = cpool.tile([128, NBB, ROWE], BF16, tag=f"gtB{i}")
                nc.vector.memset(t[:], 0.0)
                gtB.append(t)

            def recipf(out_ap, in_ap):
                if KRAF:
                    nc.vector.reciprocal_approx_fast(out=out_ap, in_=in_ap)
                else:
                    nc.vector.reciprocal(out_ap, in_ap)

            def emit_table_rows(l, t, h, wv):
                """g = h * w (bf16), plus w columns; DMA to the own-table half."""
                rows = min(128, NPC - t * 128)
                tb = tbpool.tile([128, F + H], BF16, tag="tb")
                nc.vector.tensor_tensor(
                    tb[:, 0:F].rearrange("p (h d) -> p h d", d=D),
                    h[:].rearrange("p (h d) -> p h d", d=D),
                    wv[:].to_broadcast((128, H, D)),
                    op=ALU.mult,
                )
                nc.vector.tensor_copy(tb[:, F : F + H], wv[:])
                if t < NBLKA:
                    dst = tblA_own[l][t * 128 : t * 128 + rows, 0 : F + H]
                else:
                    r0 = t * 128 - SPLITA
                    dst = tblB_own[l][r0 : r0 + rows, 0 : F + H]
                nc.sync.dma_start(out=dst, in_=tb[:rows, :])

            def att_scores(u, dst_ap):
                """dst = su + (K2/K1)*sa, where su/sa are +/- abs row sums of u."""
                su = smpool.tile([128, H], F32, tag="su")
                nc.vector.tensor_reduce(
                    su[:],
                    u[:].rearrange("p (h d) -> p h d", d=D),
                    axis=mybir.AxisListType.X,
                    op=ALU.add,
                )
                sa = smpool.tile([128, H], F32, tag="sa")
                nc.vector.tensor_reduce(
                    sa[:],
                    u[:].rearrange("p (h d) -> p h d", d=D),
                    axis=mybir.AxisListType.X,
                    op=ALU.add,
                    apply_absolute_value=True,
                )
                t1 = smpool.tile([128, H], F32, tag="t1")
                nc.vector.tensor_scalar(t1[:], sa[:], K2 / K1, None, op0=ALU.mult)
                nc.vector.tensor_tensor(dst_ap, su[:], t1[:], op=ALU.add)

            def build0(t):
                """Layer-0 GEMM + table row for destination block t."""
                ps = gpsum.tile([128, F], F32, tag="gemm")
                nc.tensor.matmul(
                    ps[:],
                    lhsT=xt_s[:, t * 128 : (t + 1) * 128],
                    rhs=w0t_s[:],
                    start=True,
                    stop=True,
                )
                h = hpool.tile([128, F], F32, tag="h")
                nc.vector.tensor_tensor(h[:], ps[:], b0b_s[:], op=ALU.add)
                u = hpool.tile([128, F], F32, tag="u")
                nc.vector.tensor_tensor(u[:], h[:], ar0_s[:], op=ALU.mult)
                srf = smpool.tile([128, H], F32, tag="srf")
                att_scores(u, srf[:])
                wv = smpool.tile([128, H], F32, tag="wv")
                nc.scalar.activation(wv[:], srf[:], AF.Exp, scale=K1)
                emit_table_rows(0, t, h, wv)

            def gather_block(l, t, qbase):
                """Issue the A/B gathers for destination block t of layer l."""
                ga = gtA[t % 3]
                gb = gtB[t % 3]
                nc.gpsimd.dma_gather(
                    ga[:],
                    tblA_full[l][:, :],
                    eidxA_s[:, t * NBA * 8 : (t + 1) * NBA * 8],
                    NBA * 128,
                    NBA * 128,
                    ROWE,
                    single_packet=(NBA * 128 <= 1024),
                    queue_num=(qbase % 4) if KQ4 else 0,
                )
                nc.gpsimd.dma_gather(
                    gb[:],
                    tblB_full[l][:, :],
                    eidxB_s[:, t * NBB * 8 : (t + 1) * NBB * 8],
                    NBB * 128,
                    NBB * 128,
                    ROWE,
                    single_packet=(NBB * 128 <= 1024),
                    queue_num=((qbase + 1) % 4) if KQ4 else 0,
                )
                return ga, gb

            def load_onehot(t):
                oh = ohpool.tile([128, NBT * 128], BF16, tag="oh")
                nc.sync.dma_start(
                    out=oh[:],
                    in_=ohm_d[:, t * NBT * 128 : (t + 1) * NBT * 128],
                )
                return oh

            def agg_matmuls(ga, gb, oh):
                ps = apsum.tile([128, F + H], F32, tag="agg")
                nmm = 0
                for r, nb, gt in ((0, NBA, ga), (1, NBB, gb)):
                    c0 = 0 if r == 0 else NBA
                    for b in range(nb):
                        nc.tensor.matmul(
                            ps[:],
                            lhsT=oh[:, (c0 + b) * 128 : (c0 + b + 1) * 128],
                            rhs=gt[:, b, 0 : F + H],
                            start=(nmm == 0),
                            stop=(nmm == NBT - 1),
                        )
                        nmm += 1
                return ps

            def agg0_gemm1(t):
                """Aggregate layer 0 for block t, then the layer-1 GEMM/scores."""
                ga, gb = gather_block(0, t, 2 * t)
                oh = load_onehot(t)
                ps = agg_matmuls(ga, gb, oh)
                rec = smpool.tile([128, H], F32, tag="rec")
                recipf(rec[:], ps[:, F : F + H])
                a0 = postpool.tile([128, F], F32, tag="a0")
                nc.vector.tensor_tensor(
                    a0[:].rearrange("p (h d) -> p h d", d=D),
                    ps[:, 0:F].rearrange("p (h d) -> p h d", d=D),
                    rec[:].to_broadcast((128, H, D)),
                    op=ALU.mult,
                )
                # LayerNorm statistics: var = (sumsq - sum^2/F) / F
                sm = smpool.tile([128, 1], F32, tag="sm")
                nc.vector.tensor_reduce(
                    sm[:], a0[:], axis=mybir.AxisListType.X, op=ALU.add
                )
                scr = postpool.tile([128, F], F32, tag="scr")
                sq = smpool.tile([128, 1], F32, tag="sq")
                nc.scalar.activation(scr[:], a0[:], AF.Square, accum_out=sq[:])
                mun = smpool.tile([128, 1], F32, tag="mun")
                nc.vector.tensor_scalar(mun[:], sm[:], -1.0 / F, None, op0=ALU.mult)
                m2 = smpool.tile([128, 1], F32, tag="m2")
                nc.vector.tensor_tensor(m2[:], mun[:], sm[:], op=ALU.mult)
                dv = smpool.tile([128, 1], F32, tag="dv")
                nc.vector.tensor_tensor(dv[:], sq[:], m2[:], op=ALU.add)
                sd = smpool.tile([128, 1], F32, tag="sd")
                nc.scalar.activation(
                    sd[:], dv[:], AF.Sqrt, bias=epsb_s[:], scale=1.0 / F
                )
                rstd = smpool.tile([128, 1], F32, tag="rstd")
                recipf(rstd[:], sd[:])
                nmr = smpool.tile([128, 1], F32, tag="nmr")
                nc.vector.tensor_tensor(nmr[:], mun[:], rstd[:], op=ALU.mult)
                if plain_ln:
                    nc.scalar.activation(
                        x1_s[:, t * F : (t + 1) * F],
                        a0[:],
                        AF.Lrelu,
                        bias=nmr[:],
                        scale=rstd[:],
                        alpha=SLOPE,
                    )
                else:
                    xn = postpool.tile([128, F], F32, tag="xn")
                    nc.scalar.activation(
                        xn[:], a0[:], AF.Copy, bias=nmr[:], scale=rstd[:]
                    )
                    xg = postpool.tile([128, F], F32, tag="xg")
                    nc.vector.tensor_tensor(xg[:], xn[:], gmb_s[:], op=ALU.mult)
                    xgb = postpool.tile([128, F], F32, tag="xgb")
                    nc.vector.tensor_tensor(xgb[:], xg[:], btb_s[:], op=ALU.add)
                    nc.scalar.activation(
                        x1_s[:, t * F : (t + 1) * F], xgb[:], AF.Lrelu, alpha=SLOPE
                    )
                # ---- layer-1 GEMM + attention scores (no exp yet) ----
                ps2 = gpsum.tile([128, F], F32, tag="gemm")
                for cch in range(2):
                    pt = tpsum.tile([128, 128], BF16, tag="tp")
                    nc.tensor.transpose(
                        pt[:],
                        x1_s[:, t * F + cch * 128 : t * F + (cch + 1) * 128],
                        idn_s[:],
                    )
                    st = spool.tile([128, 128], BF16, tag="stat")
                    nc.vector.tensor_copy(st[:], pt[:])
                    nc.tensor.matmul(
                        ps2[:],
                        lhsT=st[:],
                        rhs=w1t_s[cch][:],
                        start=(cch == 0),
                        stop=(cch == 1),
                    )
                h1 = hpool.tile([128, F], F32, tag="h")
                nc.vector.tensor_tensor(h1[:], ps2[:], b1b_s[:], op=ALU.add)
                u1 = hpool.tile([128, F], F32, tag="u")
                nc.vector.tensor_tensor(u1[:], h1[:], ar1_s[:], op=ALU.mult)
                att_scores(u1, srf1_s[:, t * H : (t + 1) * H])
                nc.vector.tensor_copy(h1b_s[:, t * F : (t + 1) * F], h1[:])

            def build1_rows(t):
                """exp + table rows for layer 1 (separate sweep: exp table)."""
                wv = smpool.tile([128, H], F32, tag="wv")
                nc.scalar.activation(
                    wv[:], srf1_s[:, t * H : (t + 1) * H], AF.Exp, scale=K1
                )
                hb = h1b_s[:, t * F : (t + 1) * F]
                rows = min(128, NPC - t * 128)
                tb = tbpool.tile([128, F + H], BF16, tag="tb")
                nc.vector.tensor_tensor(
                    tb[:, 0:F].rearrange("p (h d) -> p h d", d=D),
                    hb.rearrange("p (h d) -> p h d", d=D),
                    wv[:].to_broadcast((128, H, D)),
                    op=ALU.mult,
                )
                nc.vector.tensor_copy(tb[:, F : F + H], wv[:])
                if t < NBLKA:
                    dst = tblA_own[1][t * 128 : t * 128 + rows, 0 : F + H]
                else:
                    r0 = t * 128 - SPLITA
                    dst = tblB_own[1][r0 : r0 + rows, 0 : F + H]
                nc.sync.dma_start(out=dst, in_=tb[:rows, :])

            def agg1(t):
                """Aggregate layer 1 for block t: num/den then head mean."""
                ga, gb = gather_block(1, t, 2 * t)
                oh = load_onehot(t)
                ps = agg_matmuls(ga, gb, oh)
                rows = min(128, NPC - t * 128)
                rec = smpool.tile([128, H], F32, tag="rec")
                recipf(rec[:], ps[:, F : F + H])
                rec4 = smpool.tile([128, H], F32, tag="rec4")
                nc.vector.tensor_scalar(rec4[:], rec[:], 0.25, None, op0=ALU.mult)
                q = postpool.tile([128, F], F32, tag="a0")
                nc.vector.tensor_tensor(
                    q[:].rearrange("p (h d) -> p h d", d=D),
                    ps[:, 0:F].rearrange("p (h d) -> p h d", d=D),
                    rec4[:].to_broadcast((128, H, D)),
                    op=ALU.mult,
                )
                o = postpool.tile([128, D], F32, tag="o")
                if KSTR:
                    nc.vector.tensor_reduce(
                        o[:],
                        q[:].rearrange("p (h d) -> p d h", d=D),
                        axis=mybir.AxisListType.X,
                        op=ALU.add,
                    )
                else:
                    p01 = postpool.tile([128, D], F32, tag="p01")
                    nc.vector.tensor_tensor(
                        p01[:], q[:, 0:D], q[:, D : 2 * D], op=ALU.add
                    )
                    p23 = postpool.tile([128, D], F32, tag="p23")
                    nc.vector.tensor_tensor(
                        p23[:], q[:, 2 * D : 3 * D], q[:, 3 * D : 4 * D], op=ALU.add
                    )
                    nc.vector.tensor_tensor(o[:], p01[:], p23[:], op=ALU.add)
                nc.sync.dma_start(
                    out=out_d[t * 128 : t * 128 + rows, :], in_=o[:rows, :]
                )

            def allgather(l, half):
                if half == 0:
                    nc.gpsimd.collective_compute(
                        "AllGather",
                        ALU.bypass,
                        replica_groups=groups,
                        ins=[tblA_own[l][:, :]],
                        outs=[tblA_full[l][:, :]],
                    )
                else:
                    nc.gpsimd.collective_compute(
                        "AllGather",
                        ALU.bypass,
                        replica_groups=groups,
                        ins=[tblB_own[l][:, :]],
                        outs=[tblB_full[l][:, :]],
                    )

            # ================= schedule =================
            if "b0" in KPH:
                for t in range(NBLKA):
                    build0(t)
                allgather(0, 0)
                for t in range(NBLKA, NBLK):
                    build0(t)
                allgather(0, 1)
            if "a0" in KPH:
                for t in range(NBLK):
                    agg0_gemm1(t)
            if "b1" in KPH:
                for t in range(NBLKA):
                    build1_rows(t)
                allgather(1, 0)
                for t in range(NBLKA, NBLK):
                    build1_rows(t)
                allgather(1, 1)
            if "a1" in KPH:
                for t in range(NBLK):
                    agg1(t)
            else:
                zt = postpool.tile([128, D], F32, tag="o")
                nc.vector.memset(zt[:], 0.0)
                nc.sync.dma_start(out=out_d[0:128, :], in_=zt[:])

    nc.compile()
    return nc


_CACHE = {}


def kernel(**inputs):
    global LAST_RESULTS
    in_maps, NBA, NBB, plain_ln = _host_prep(**inputs)
    key = (NBA, NBB, plain_ln, os.environ.get("KQ4"), os.environ.get("KRAF"), os.environ.get("KSTR"), os.environ.get("KPH"))
    if key not in _CACHE:
        _CACHE[key] = _build_program(NBA, NBB, plain_ln)
    nc = _CACHE[key]
    trace = bool(os.environ.get("BASS_TRACE"))
    res = run_bass_kernel_spmd(nc, in_maps, list(range(NCORE)), trace=trace)
    LAST_RESULTS = res
    out = np.concatenate([res.results[c]["out"] for c in range(NCORE)], axis=0)
    return out.astype(np.float32)
